# revision 8
# baseline (speedup 1.0000x reference)
"""Trainium2 Bass kernel for nn_Decoder_73194832658812.

Single-step attention decoder:
  word-embedding lookup + Bahdanau additive attention over encoder memory
  + GRUCell step + vocab projection.

Sharding (8 NeuronCores):
  - data-parallel over batch (8 rows/core) for attention + GRU
  - AllGather of new_hidden (transposed, bf16), then vocab-sharded
    out-projection (each core computes 4000 of the 32000 logits for all 64
    batch rows).

All matmuls run in bf16 with fp32 PSUM accumulation; softmax / GRU
combine arithmetic stays fp32.
"""

import numpy as np
import ml_dtypes

BF16 = ml_dtypes.bfloat16

# problem dims (hardcoded per spec)
B, S, M, H, EMB, V = 64, 512, 2048, 1024, 512, 32000
NCORES = 8
BPC = B // NCORES          # 8 batch rows per core
VS = V // NCORES           # 4000 vocab columns per core
GH = 3 * H                 # 3072 GRU gate width
XK = M + EMB               # 2560 GRU input width
P = 128
MT, HT, ET, ST = M // P, H // P, EMB // P, S // P   # 16, 8, 4, 4
KT = MT + ET               # 20 k-tiles of GRU input

_CACHE = {}


def _build_program():
    """Build + compile the per-core Bass program (SPMD: same program on all
    8 cores, per-core data differs)."""
    import concourse.bass as bass
    from concourse import bacc, mybir, tile
    from concourse.bass import ts, ds
    from concourse.masks import make_identity

    dt = mybir.dt
    FP32, BF = dt.float32, dt.bfloat16
    TANH = mybir.ActivationFunctionType.Tanh
    SIGM = mybir.ActivationFunctionType.Sigmoid
    EXP = mybir.ActivationFunctionType.Exp

    nc = bacc.Bacc(
        "TRN2",
        target_bir_lowering=False,
        debug=False,
        enable_asserts=False,
        num_devices=NCORES,
    )

    # ---- per-core DRAM I/O ----
    d_enc = nc.dram_tensor("enc", [BPC, S, M], BF, kind="ExternalInput")
    d_hidT = nc.dram_tensor("hidT", [H, BPC], BF, kind="ExternalInput")
    d_hidf = nc.dram_tensor("hidf", [BPC, H], FP32, kind="ExternalInput")
    d_wvT = nc.dram_tensor("wvT", [EMB, BPC], BF, kind="ExternalInput")
    d_lin1wT = nc.dram_tensor("lin1wT", [H, H], BF, kind="ExternalInput")
    d_lin2wT = nc.dram_tensor("lin2wT", [M, H], BF, kind="ExternalInput")
    d_hxb = nc.dram_tensor("hxb", [P, HT], FP32, kind="ExternalInput")
    d_vcolT = nc.dram_tensor("vcolT", [P, HT], BF, kind="ExternalInput")
    d_WihT = nc.dram_tensor("WihT", [XK, GH], BF, kind="ExternalInput")
    d_WhhT = nc.dram_tensor("WhhT", [H, GH], BF, kind="ExternalInput")
    d_brz = nc.dram_tensor("brz", [1, 2 * H], FP32, kind="ExternalInput")
    d_bin = nc.dram_tensor("bin", [1, H], FP32, kind="ExternalInput")
    d_bhn = nc.dram_tensor("bhn", [1, H], FP32, kind="ExternalInput")
    d_outwT = nc.dram_tensor("outwT", [H, VS], BF, kind="ExternalInput")
    d_outb = nc.dram_tensor("outb", [1, VS], FP32, kind="ExternalInput")

    d_pred = nc.dram_tensor("pred", [B, VS], FP32, kind="ExternalOutput")
    d_nh = nc.dram_tensor("nh", [BPC, H], FP32, kind="ExternalOutput")
    d_score = nc.dram_tensor("score", [BPC, S], FP32, kind="ExternalOutput")

    with tile.TileContext(nc) as tc:
        with (
            tc.tile_pool(name="const", bufs=1) as cst,
            tc.tile_pool(name="wres", bufs=1) as wres,
            tc.tile_pool(name="dram", bufs=1, space="DRAM") as drp,
        ):
            # ---- resident constants / weights ----
            ident_bf = cst.tile([P, P], BF, tag="ident_bf")
            make_identity(nc, ident_bf[:])
            ident_f = cst.tile([P, P], FP32, tag="ident_f")
            make_identity(nc, ident_f[:])

            lin2w_sb = wres.tile([P, MT, H], BF, tag="lin2w")
            nc.sync.dma_start(
                lin2w_sb[:], d_lin2wT.ap().rearrange("(mt p) h -> p mt h", p=P)
            )
            hidT_sb = cst.tile([P, HT, BPC], BF, tag="hidT")
            nc.sync.dma_start(
                hidT_sb[:], d_hidT.ap().rearrange("(t p) b -> p t b", p=P)
            )
            hxb_sb = cst.tile([P, HT], FP32, tag="hxb")
            nc.sync.dma_start(hxb_sb[:], d_hxb.ap())
            vcolT_sb = cst.tile([P, HT], BF, tag="vcolT")
            nc.sync.dma_start(vcolT_sb[:], d_vcolT.ap())
            # hxbias[h, t*BPC + b] = (hidden @ lin1_w.T)[b, t*128+h_p] + (lin1_b+lin2_b)
            hxbias_sb = cst.tile([P, HT * BPC], FP32, tag="hxbias")

            # ---- prologue: h_x = hidden @ lin1_w.T (transposed + bias) ----
            with (
                tc.tile_pool(name="pro", bufs=1) as pro,
                tc.tile_pool(name="pro_ps", bufs=2, space="PSUM") as pro_ps,
            ):
                lin1w_sb = pro.tile([P, HT, H], BF, tag="lin1w")
                nc.sync.dma_start(
                    lin1w_sb[:], d_lin1wT.ap().rearrange("(t p) h -> p t h", p=P)
                )
                hx_sb = pro.tile([BPC, H], FP32, tag="hx")
                for ch in range(2):
                    hx_ps = pro_ps.tile([BPC, 512], FP32, tag="hx_ps")
                    for kt in range(HT):
                        nc.tensor.matmul(
                            hx_ps[:],
                            hidT_sb[:, kt, :],
                            lin1w_sb[:, kt, ds(ch * 512, 512)],
                            start=(kt == 0),
                            stop=(kt == HT - 1),
                        )
                    nc.vector.tensor_copy(hx_sb[:, ds(ch * 512, 512)], hx_ps[:])
                for t in range(HT):
                    tp_ps = pro_ps.tile([P, BPC], FP32, tag="tp_ps")
                    nc.tensor.transpose(
                        tp_ps[:], hx_sb[:, ds(t * P, P)], ident_f[:BPC, :BPC]
                    )
                    nc.vector.tensor_add(
                        hxbias_sb[:, ds(t * BPC, BPC)],
                        tp_ps[:],
                        hxb_sb[:, t : t + 1].broadcast_to([P, BPC]),
                    )

            # ---- phase 1: attention (per batch row) ----
            xT_sb = cst.tile([P, KT, BPC], BF, tag="xT")  # GRU input, transposed
            nc.sync.dma_start(
                xT_sb[:, MT:KT, :], d_wvT.ap().rearrange("(t p) b -> p t b", p=P)
            )

            with (
                tc.tile_pool(name="encT", bufs=2) as encTp,
                tc.tile_pool(name="encN", bufs=2) as encNp,
                tc.tile_pool(name="tanh", bufs=2) as tanhp,
                tc.tile_pool(name="attn_sm", bufs=2) as smp,
                tc.tile_pool(name="hm_ps", bufs=4, space="PSUM") as hm_ps,
                tc.tile_pool(name="e_ps", bufs=2, space="PSUM") as e_ps,
                tc.tile_pool(name="ctx_ps", bufs=1, space="PSUM") as ctx_psp,
                tc.tile_pool(name="tr_ps", bufs=1, space="PSUM") as tr_ps,
            ):
                ctx_ps = ctx_psp.tile([P, MT * BPC], FP32, tag="ctx")

                for b in range(BPC):
                    # transposed encoder tiles [m_p, mt, s] via DMA-transpose
                    encT = encTp.tile([P, MT, S], BF, tag="encT")
                    for mt in range(MT):
                        nc.sync.dma_start_transpose(
                            encT[:, mt, :], d_enc.ap()[b, :, ds(mt * P, P)]
                        )
                    # natural encoder tiles [s_p, st, m] (for context matmul)
                    encN = encNp.tile([P, ST, M], BF, tag="encN")
                    nc.sync.dma_start(
                        encN[:], d_enc.ap()[b].rearrange("(st p) m -> p st m", p=P)
                    )

                    # h_mT[h, s] = lin2_w @ enc[b].T ; t = tanh(h_mT + h_x[b] + bias)
                    t_sb = tanhp.tile([P, HT, S], BF, tag="t_sb")
                    e_psum = e_ps.tile([1, S], FP32, tag="e")
                    for ht in range(HT):
                        hm = hm_ps.tile([P, S], FP32, tag="hm")
                        for mt in range(MT):
                            nc.tensor.matmul(
                                hm[:],
                                lin2w_sb[:, mt, ds(ht * P, P)],
                                encT[:, mt, :],
                                start=(mt == 0),
                                stop=(mt == MT - 1),
                            )
                        nc.scalar.activation(
                            t_sb[:, ht, :],
                            hm[:],
                            TANH,
                            bias=hxbias_sb[:, ht * BPC + b : ht * BPC + b + 1],
                        )
                        # e[s] += v[ht*128:...] . t[ht]
                        nc.tensor.matmul(
                            e_psum[:],
                            vcolT_sb[:, ht : ht + 1],
                            t_sb[:, ht, :],
                            start=(ht == 0),
                            stop=(ht == HT - 1),
                        )

                    # softmax over s (single partition, 512 wide)
                    negmax = smp.tile([1, 1], FP32, tag="negmax")
                    nc.vector.tensor_reduce(
                        negmax[:], e_psum[:], axis=mybir.AxisListType.X,
                        op=mybir.AluOpType.max, negate=True,
                    )
                    prob = smp.tile([1, S], FP32, tag="prob")
                    ssum = smp.tile([1, 1], FP32, tag="ssum")
                    nc.scalar.activation(
                        prob[:], e_psum[:], EXP, bias=negmax[:], accum_out=ssum[:]
                    )
                    rinv = smp.tile([1, 1], FP32, tag="rinv")
                    nc.vector.reciprocal(rinv[:], ssum[:])
                    score_sb = smp.tile([1, S], FP32, tag="score")
                    nc.vector.tensor_scalar_mul(score_sb[:], prob[:], rinv[:])
                    nc.sync.dma_start(d_score.ap()[b : b + 1, :], score_sb[:])

                    # score column tiles (bf16) via PE transpose
                    score_bf = smp.tile([1, S], BF, tag="score_bf")
                    nc.vector.tensor_copy(score_bf[:], score_sb[:])
                    score_colT = smp.tile([P, ST], BF, tag="score_colT")
                    for st in range(ST):
                        trp = tr_ps.tile([P, 1], BF, tag="trp")
                        nc.tensor.transpose(
                            trp[:], score_bf[:, ds(st * P, P)], ident_bf[:1, :1]
                        )
                        nc.vector.tensor_copy(score_colT[:, st : st + 1], trp[:])

                    # contextT[m, b] = sum_s enc[b, s, m] * score[s]
                    for mt in range(MT):
                        col = mt * BPC + b
                        for st in range(ST):
                            nc.tensor.matmul(
                                ctx_ps[:, col : col + 1],
                                encN[:, st, ds(mt * P, P)],
                                score_colT[:, st : st + 1],
                                start=(st == 0),
                                stop=(st == ST - 1),
                            )

                # xT[:, 0:MT, :] = contextT (cast to bf16)
                nc.vector.tensor_copy(
                    xT_sb[:, 0:MT, :].rearrange("p t b -> p (t b)"), ctx_ps[:]
                )

            # ---- phase 2: GRU cell (data-parallel, activation-stationary) ----
            nhT_dram = drp.tile([H, BPC], BF, tag="nhT_in")
            nhT_all_dram = drp.tile([NCORES * H, BPC], BF, tag="nhT_all",
                                    addr_space="Shared")

            with (
                tc.tile_pool(name="gr_sb", bufs=2) as grs,
                tc.tile_pool(name="gr_cst", bufs=1) as grc,
                tc.tile_pool(name="gruw", bufs=6) as gruw,
                tc.tile_pool(name="g_ps", bufs=1, space="PSUM") as g_ps,
                tc.tile_pool(name="ntr_ps", bufs=2, space="PSUM") as ntr_ps,
            ):
                hidf_sb = grc.tile([BPC, H], FP32, tag="hidf")
                nc.sync.dma_start(hidf_sb[:], d_hidf.ap())
                # row-vector biases replicated across partitions via DMA
                brz_sb = grc.tile([BPC, 2 * H], FP32, tag="brz")
                nc.sync.dma_start(brz_sb[:], d_brz.ap().broadcast_to([BPC, 2 * H]))
                bin_sb = grc.tile([BPC, H], FP32, tag="bin")
                nc.sync.dma_start(bin_sb[:], d_bin.ap().broadcast_to([BPC, H]))
                bhn_sb = grc.tile([BPC, H], FP32, tag="bhn")
                nc.sync.dma_start(bhn_sb[:], d_bhn.ap().broadcast_to([BPC, H]))
                nh_sb = grs.tile([BPC, H], FP32, tag="nh_sb")
                nhbf_sb = grs.tile([BPC, H], BF, tag="nhbf")
                for ch in range(2):
                    co = ch * 512  # column offset within each gate block
                    rps = g_ps.tile([BPC, 512], FP32, tag="rps")
                    zps = g_ps.tile([BPC, 512], FP32, tag="zps")
                    inps = g_ps.tile([BPC, 512], FP32, tag="inps")
                    hnps = g_ps.tile([BPC, 512], FP32, tag="hnps")
                    for kt in range(KT):
                        gw = gruw.tile([P, 3, 512], BF, tag="gw")
                        nc.sync.dma_start(
                            gw[:],
                            d_WihT.ap()[ts(kt, P)]
                            .rearrange("p (g c) -> p g c", g=3)[:, :, ds(co, 512)],
                        )
                        lhs = xT_sb[:, kt, :]
                        nc.tensor.matmul(rps[:], lhs, gw[:, 0, :],
                                         start=(kt == 0), stop=False)
                        nc.tensor.matmul(zps[:], lhs, gw[:, 1, :],
                                         start=(kt == 0), stop=False)
                        nc.tensor.matmul(inps[:], lhs, gw[:, 2, :],
                                         start=(kt == 0), stop=(kt == KT - 1))
                    for kt in range(HT):
                        hw = gruw.tile([P, 3, 512], BF, tag="gw")
                        nc.sync.dma_start(
                            hw[:],
                            d_WhhT.ap()[ts(kt, P)]
                            .rearrange("p (g c) -> p g c", g=3)[:, :, ds(co, 512)],
                        )
                        lhs = hidT_sb[:, kt, :]
                        nc.tensor.matmul(rps[:], lhs, hw[:, 0, :],
                                         start=False, stop=(kt == HT - 1))
                        nc.tensor.matmul(zps[:], lhs, hw[:, 1, :],
                                         start=False, stop=(kt == HT - 1))
                        nc.tensor.matmul(hnps[:], lhs, hw[:, 2, :],
                                         start=(kt == 0), stop=(kt == HT - 1))

                    # r = sigm(rps + brz[ch]); z = sigm(zps + brz[2H..]);
                    # n = tanh(inps + bin + r * (hnps + bhn))
                    r_sb = grs.tile([BPC, 512], FP32, tag="r_sb")
                    nc.vector.tensor_add(r_sb[:], rps[:], brz_sb[:, ds(co, 512)])
                    nc.scalar.activation(r_sb[:], r_sb[:], SIGM)
                    z_sb = grs.tile([BPC, 512], FP32, tag="z_sb")
                    nc.vector.tensor_add(z_sb[:], zps[:], brz_sb[:, ds(H + co, 512)])
                    nc.scalar.activation(z_sb[:], z_sb[:], SIGM)
                    hn_sb = grs.tile([BPC, 512], FP32, tag="hn_sb")
                    nc.vector.tensor_add(hn_sb[:], hnps[:], bhn_sb[:, ds(co, 512)])
                    nc.vector.tensor_mul(hn_sb[:], hn_sb[:], r_sb[:])
                    in_sb = grs.tile([BPC, 512], FP32, tag="in_sb")
                    nc.vector.tensor_add(in_sb[:], inps[:], bin_sb[:, ds(co, 512)])
                    nc.vector.tensor_add(in_sb[:], in_sb[:], hn_sb[:])
                    n_sb = grs.tile([BPC, 512], FP32, tag="n_sb")
                    nc.scalar.activation(n_sb[:], in_sb[:], TANH)
                    # nh = n + z * (hidden - n)
                    dd = grs.tile([BPC, 512], FP32, tag="dd")
                    nc.vector.tensor_sub(dd[:], hidf_sb[:, ds(co, 512)], n_sb[:])
                    nc.vector.tensor_mul(dd[:], dd[:], z_sb[:])
                    nc.vector.tensor_add(nh_sb[:, ds(co, 512)], n_sb[:], dd[:])

                nc.sync.dma_start(d_nh.ap(), nh_sb[:])
                nc.vector.tensor_copy(nhbf_sb[:], nh_sb[:])
                # transpose nh -> [H, BPC] bf16 and stage to DRAM for AllGather
                nhT_sb = grs.tile([P, HT, BPC], BF, tag="nhT_sb")
                for t in range(HT):
                    ntp = ntr_ps.tile([P, BPC], BF, tag="ntp")
                    nc.tensor.transpose(
                        ntp[:], nhbf_sb[:, ds(t * P, P)], ident_bf[:BPC, :BPC]
                    )
                    nc.vector.tensor_copy(nhT_sb[:, t, :], ntp[:])
                nc.sync.dma_start(
                    nhT_dram[:].rearrange("(t p) b -> p t b", p=P), nhT_sb[:]
                )

            nc.gpsimd.collective_compute(
                "AllGather",
                mybir.AluOpType.bypass,
                replica_groups=[list(range(NCORES))],
                ins=[nhT_dram.opt()],
                outs=[nhT_all_dram.opt()],
            )

            # ---- phase 3: vocab-sharded out projection ----
            with (
                tc.tile_pool(name="op_sb", bufs=2) as ops,
                tc.tile_pool(name="outw", bufs=8) as outw,
                tc.tile_pool(name="op_ps", bufs=4, space="PSUM") as op_ps,
            ):
                outb_sb = ops.tile([B, VS], FP32, tag="outb")
                nc.sync.dma_start(outb_sb[:], d_outb.ap().broadcast_to([B, VS]))
                nhT_all_sb = ops.tile([P, HT, NCORES, BPC], BF, tag="nhT_all")
                for r in range(NCORES):
                    nc.sync.dma_start(
                        nhT_all_sb[:, :, r, :],
                        nhT_all_dram[ds(r * H, H), :]
                        .rearrange("(t p) b -> p t b", p=P),
                    )
                ow_tiles = []
                for kt in range(HT):
                    owt = outw.tile([P, VS], BF, tag="owt")
                    nc.sync.dma_start(owt[:], d_outwT.ap()[ts(kt, P)])
                    ow_tiles.append(owt)
                NCH = 8
                CW = VS // NCH  # 500
                for c in range(NCH):
                    pps = op_ps.tile([B, CW], FP32, tag="pps")
                    for kt in range(HT):
                        nc.tensor.matmul(
                            pps[:],
                            nhT_all_sb[:, kt, :, :],
                            ow_tiles[kt][:, ds(c * CW, CW)],
                            start=(kt == 0),
                            stop=(kt == HT - 1),
                        )
                    psb = ops.tile([B, CW], FP32, tag="psb")
                    nc.vector.tensor_add(psb[:], pps[:], outb_sb[:, ds(c * CW, CW)])
                    nc.sync.dma_start(d_pred.ap()[:, ds(c * CW, CW)], psb[:])

    nc.compile()
    return nc


def _bf(x):
    return np.ascontiguousarray(np.asarray(x, np.float32)).astype(BF16)


def _prep_in_maps(inputs):
    """Host-side prep: dtype casts, transposes, sharding. Returns per-core
    input dicts keyed by the kernel's DRAM tensor names."""
    f32 = lambda k: np.asarray(inputs[k], np.float32)
    word = np.asarray(inputs["word"]).astype(np.int64)
    emb = f32("emb")
    hidden = f32("hidden")
    enc = f32("encoder_outputs")

    word_vec = emb[word]                                   # [B, EMB] host gather
    enc_bf = _bf(enc)                                      # [B, S, M]
    hidT_bf = _bf(hidden.T)                                # [H, B]
    wvT_bf = _bf(word_vec.T)                               # [EMB, B]
    lin1wT = _bf(f32("lin1_w").T)
    lin2wT = _bf(f32("lin2_w").T)
    hxb = np.ascontiguousarray(
        (f32("lin1_b") + f32("lin2_b")).reshape(HT, P).T
    )                                                      # [128, HT]
    vcolT = _bf(f32("v_w")[0].reshape(HT, P).T)            # [128, HT]
    WihT = _bf(f32("W_ih").T)
    WhhT = _bf(f32("W_hh").T)
    b_ih, b_hh = f32("b_ih"), f32("b_hh")
    brz = np.ascontiguousarray((b_ih[: 2 * H] + b_hh[: 2 * H])[None, :])
    b_in = np.ascontiguousarray(b_ih[2 * H :][None, :])
    b_hn = np.ascontiguousarray(b_hh[2 * H :][None, :])
    outwT = _bf(f32("out_w").T)                            # [H, V]
    outb = f32("out_b")

    in_maps = []
    for c in range(NCORES):
        bs = slice(c * BPC, (c + 1) * BPC)
        vs = slice(c * VS, (c + 1) * VS)
        in_maps.append(
            {
                "enc": np.ascontiguousarray(enc_bf[bs]),
                "hidT": np.ascontiguousarray(hidT_bf[:, bs]),
                "hidf": np.ascontiguousarray(hidden[bs]),
                "wvT": np.ascontiguousarray(wvT_bf[:, bs]),
                "lin1wT": lin1wT,
                "lin2wT": lin2wT,
                "hxb": hxb,
                "vcolT": vcolT,
                "WihT": WihT,
                "WhhT": WhhT,
                "brz": brz,
                "bin": b_in,
                "bhn": b_hn,
                "outwT": np.ascontiguousarray(outwT[:, vs]),
                "outb": np.ascontiguousarray(outb[None, vs]),
            }
        )
    return in_maps


def _assemble(results):
    pred = np.concatenate([r["pred"] for r in results], axis=1)
    nh = np.concatenate([r["nh"] for r in results], axis=0)
    score = np.concatenate([r["score"] for r in results], axis=0)
    return (
        np.ascontiguousarray(pred, dtype=np.float32),
        np.ascontiguousarray(nh, dtype=np.float32),
        np.ascontiguousarray(score, dtype=np.float32),
    )


def kernel(**inputs):
    from concourse import bass_utils

    if "nc" not in _CACHE:
        _CACHE["nc"] = _build_program()
    nc = _CACHE["nc"]
    in_maps = _prep_in_maps(inputs)
    res = bass_utils.run_bass_kernel_spmd(
        nc, in_maps, core_ids=list(range(NCORES))
    )
    return _assemble(res.results)


# revision 11
# speedup vs baseline: 1.0868x; 1.0868x over previous
"""Trainium2 Bass kernel for nn_Decoder_73194832658812.

Single-step attention decoder:
  word-embedding lookup + Bahdanau additive attention over encoder memory
  + GRUCell step + vocab projection.

Sharding (8 NeuronCores):
  - data-parallel over batch (8 rows/core) for the attention phase
  - AllGather of context (transposed, bf16), then H-sharded GRU (each core
    computes a 128-wide slice of new_hidden for all 64 rows)
  - AllGather of new_hidden slices, then vocab-sharded out-projection
    (each core computes 4000 of the 32000 logits for all 64 batch rows).

All matmuls run in bf16 with fp32 PSUM accumulation; softmax / GRU combine
arithmetic stays fp32. The attention batch loop is software-pipelined so the
PE stream never waits on the DVE/ACT softmax chain.
"""

import numpy as np
import ml_dtypes

BF16 = ml_dtypes.bfloat16

# problem dims (hardcoded per spec)
B, S, M, H, EMB, V = 64, 512, 2048, 1024, 512, 32000
NCORES = 8
BPC = B // NCORES          # 8 batch rows per core
VS = V // NCORES           # 4000 vocab columns per core
HS = H // NCORES           # 128 hidden slice per core (GRU shard)
XK = M + EMB               # 2560 GRU input width
P = 128
MT, HT, ET, ST = M // P, H // P, EMB // P, S // P   # 16, 8, 4, 4
KT = MT + ET               # 20 k-tiles of GRU input

_CACHE = {}


def _build_program():
    """Build + compile the per-core Bass program (SPMD: same program on all
    8 cores, per-core data differs)."""
    import concourse.bass as bass
    from concourse import bacc, mybir, tile
    from concourse.bass import ts, ds
    from concourse.masks import make_identity

    dt = mybir.dt
    FP32, BF = dt.float32, dt.bfloat16
    TANH = mybir.ActivationFunctionType.Tanh
    SIGM = mybir.ActivationFunctionType.Sigmoid
    EXP = mybir.ActivationFunctionType.Exp

    nc = bacc.Bacc(
        "TRN2",
        target_bir_lowering=False,
        debug=False,
        enable_asserts=False,
        num_devices=NCORES,
    )

    # ---- per-core DRAM I/O ----
    d_enc = nc.dram_tensor("enc", [BPC, S, M], BF, kind="ExternalInput")
    d_hidT = nc.dram_tensor("hidT", [H, BPC], BF, kind="ExternalInput")
    d_hidTF = nc.dram_tensor("hidTF", [H, B], BF, kind="ExternalInput")
    d_hidfS = nc.dram_tensor("hidfS", [B, HS], FP32, kind="ExternalInput")
    d_wvTF = nc.dram_tensor("wvTF", [EMB, B], BF, kind="ExternalInput")
    d_lin1wT = nc.dram_tensor("lin1wT", [H, H], BF, kind="ExternalInput")
    d_lin2wT = nc.dram_tensor("lin2wT", [M, H], BF, kind="ExternalInput")
    d_hxb = nc.dram_tensor("hxb", [P, HT], FP32, kind="ExternalInput")
    d_vcolT = nc.dram_tensor("vcolT", [P, HT], BF, kind="ExternalInput")
    d_WihS = nc.dram_tensor("WihS", [XK, 3 * HS], BF, kind="ExternalInput")
    d_WhhS = nc.dram_tensor("WhhS", [H, 3 * HS], BF, kind="ExternalInput")
    d_gb = nc.dram_tensor("gb", [1, 4 * HS], FP32, kind="ExternalInput")
    d_outwT = nc.dram_tensor("outwT", [H, VS], BF, kind="ExternalInput")
    d_outb = nc.dram_tensor("outb", [1, VS], FP32, kind="ExternalInput")

    d_pred = nc.dram_tensor("pred", [B, VS], FP32, kind="ExternalOutput")
    d_nh = nc.dram_tensor("nh", [B, HS], FP32, kind="ExternalOutput")
    d_score = nc.dram_tensor("score", [BPC, S], FP32, kind="ExternalOutput")

    with tile.TileContext(nc) as tc:
        with (
            tc.tile_pool(name="const", bufs=1) as cst,
            tc.tile_pool(name="wres", bufs=1) as wres,
            tc.tile_pool(name="dram", bufs=1, space="DRAM") as drp,
        ):
            ident_bf = cst.tile([P, P], BF, tag="ident_bf")
            make_identity(nc, ident_bf[:])
            ident_f = cst.tile([P, P], FP32, tag="ident_f")
            make_identity(nc, ident_f[:])

            lin2w_sb = wres.tile([P, MT, H], BF, tag="lin2w")
            nc.sync.dma_start(
                lin2w_sb[:], d_lin2wT.ap().rearrange("(mt p) h -> p mt h", p=P)
            )
            hidT_sb = cst.tile([P, HT, BPC], BF, tag="hidT")
            nc.sync.dma_start(
                hidT_sb[:], d_hidT.ap().rearrange("(t p) b -> p t b", p=P)
            )
            hxb_sb = cst.tile([P, HT], FP32, tag="hxb")
            nc.sync.dma_start(hxb_sb[:], d_hxb.ap())
            vcolT_sb = cst.tile([P, HT], BF, tag="vcolT")
            nc.sync.dma_start(vcolT_sb[:], d_vcolT.ap())
            hxbias_sb = cst.tile([P, HT * BPC], FP32, tag="hxbias")

            # DRAM staging for the two AllGathers
            ctxT_dram = drp.tile([M, BPC], BF, tag="ctxT_in")
            ctxT_all_dram = drp.tile([NCORES * M, BPC], BF, tag="ctxT_all",
                                     addr_space="Shared")
            nhT_dram = drp.tile([HS, B], BF, tag="nhT_in")
            nhT_all_dram = drp.tile([NCORES * HS, B], BF, tag="nhT_all",
                                    addr_space="Shared")

            # ---- prologue: h_xT = (hidden @ lin1_w.T).T + (lin1_b+lin2_b) ----
            with (
                tc.tile_pool(name="pro", bufs=1) as pro,
                tc.tile_pool(name="pro_ps", bufs=2, space="PSUM") as pro_ps,
            ):
                lin1w_sb = pro.tile([P, HT, H], BF, tag="lin1w")
                nc.sync.dma_start(
                    lin1w_sb[:], d_lin1wT.ap().rearrange("(t p) h -> p t h", p=P)
                )
                hx_sb = pro.tile([BPC, H], FP32, tag="hx")
                for ch in range(2):
                    hx_ps = pro_ps.tile([BPC, 512], FP32, tag="hx_ps")
                    for kt in range(HT):
                        nc.tensor.matmul(
                            hx_ps[:],
                            hidT_sb[:, kt, :],
                            lin1w_sb[:, kt, ds(ch * 512, 512)],
                            start=(kt == 0),
                            stop=(kt == HT - 1),
                        )
                    nc.vector.tensor_copy(hx_sb[:, ds(ch * 512, 512)], hx_ps[:])
                for t in range(HT):
                    tp_ps = pro_ps.tile([P, BPC], FP32, tag="tp_ps")
                    nc.tensor.transpose(
                        tp_ps[:], hx_sb[:, ds(t * P, P)], ident_f[:BPC, :BPC]
                    )
                    nc.vector.tensor_add(
                        hxbias_sb[:, ds(t * BPC, BPC)],
                        tp_ps[:],
                        hxb_sb[:, t : t + 1].broadcast_to([P, BPC]),
                    )

            # ---- phase 1: attention, software-pipelined over batch rows ----
            # stage A(b):   DMA encT/encN, h_m matmuls, tanh
            # stage B(b-1): e matvec, softmax chain
            # stage C(b-2): score transposes, context matmuls
            with (
                tc.tile_pool(name="encT", bufs=2) as encTp,
                tc.tile_pool(name="encN", bufs=3) as encNp,
                tc.tile_pool(name="tanh", bufs=2) as tanhp,
                tc.tile_pool(name="attn_sm", bufs=3) as smp,
                tc.tile_pool(name="hm_ps", bufs=4, space="PSUM") as hm_ps,
                tc.tile_pool(name="e_ps", bufs=2, space="PSUM") as e_ps,
                tc.tile_pool(name="ctx_ps", bufs=1, space="PSUM") as ctx_psp,
                tc.tile_pool(name="tr_ps", bufs=1, space="PSUM") as tr_ps,
            ):
                ctx_ps = ctx_psp.tile([P, MT * BPC], FP32, tag="ctx")
                stA, stB = {}, {}

                for step in range(BPC + 2):
                    if step < BPC:
                        b = step
                        encT = encTp.tile([P, MT, S], BF, tag="encT")
                        for mt in range(MT):
                            nc.sync.dma_start_transpose(
                                encT[:, mt, :], d_enc.ap()[b, :, ds(mt * P, P)]
                            )
                        encN = encNp.tile([P, ST, M], BF, tag="encN")
                        nc.sync.dma_start(
                            encN[:],
                            d_enc.ap()[b].rearrange("(st p) m -> p st m", p=P),
                        )
                        t_sb = tanhp.tile([P, HT, S], BF, tag="t_sb")
                        for ht in range(HT):
                            hm = hm_ps.tile([P, S], FP32, tag="hm")
                            for mt in range(MT):
                                nc.tensor.matmul(
                                    hm[:],
                                    lin2w_sb[:, mt, ds(ht * P, P)],
                                    encT[:, mt, :],
                                    start=(mt == 0),
                                    stop=(mt == MT - 1),
                                )
                            nc.scalar.activation(
                                t_sb[:, ht, :],
                                hm[:],
                                TANH,
                                bias=hxbias_sb[:, ht * BPC + b : ht * BPC + b + 1],
                            )
                        stA[b] = (encN, t_sb)

                    if 1 <= step <= BPC:
                        p_ = step - 1
                        encN, t_sb = stA.pop(p_)
                        e_psum = e_ps.tile([1, S], FP32, tag="e")
                        for ht in range(HT):
                            nc.tensor.matmul(
                                e_psum[:],
                                vcolT_sb[:, ht : ht + 1],
                                t_sb[:, ht, :],
                                start=(ht == 0),
                                stop=(ht == HT - 1),
                            )
                        negmax = smp.tile([1, 1], FP32, tag="negmax")
                        nc.vector.tensor_reduce(
                            negmax[:], e_psum[:], axis=mybir.AxisListType.X,
                            op=mybir.AluOpType.max, negate=True,
                        )
                        prob = smp.tile([1, S], FP32, tag="prob")
                        ssum = smp.tile([1, 1], FP32, tag="ssum")
                        nc.scalar.activation(
                            prob[:], e_psum[:], EXP, bias=negmax[:], accum_out=ssum[:]
                        )
                        rinv = smp.tile([1, 1], FP32, tag="rinv")
                        nc.vector.reciprocal(rinv[:], ssum[:])
                        score_sb = smp.tile([1, S], FP32, tag="score")
                        nc.vector.tensor_scalar_mul(score_sb[:], prob[:], rinv[:])
                        nc.sync.dma_start(d_score.ap()[p_ : p_ + 1, :], score_sb[:])
                        score_bf = smp.tile([1, S], BF, tag="score_bf")
                        nc.vector.tensor_copy(score_bf[:], score_sb[:])
                        stB[p_] = (encN, score_bf)

                    if step >= 2:
                        q = step - 2
                        encN, score_bf = stB.pop(q)
                        score_colT = smp.tile([P, ST], BF, tag="score_colT")
                        for st in range(ST):
                            trp = tr_ps.tile([P, 1], BF, tag="trp")
                            nc.tensor.transpose(
                                trp[:], score_bf[:, ds(st * P, P)], ident_bf[:1, :1]
                            )
                            nc.vector.tensor_copy(score_colT[:, st : st + 1], trp[:])
                        for mt in range(MT):
                            col = mt * BPC + q
                            for st in range(ST):
                                nc.tensor.matmul(
                                    ctx_ps[:, col : col + 1],
                                    encN[:, st, ds(mt * P, P)],
                                    score_colT[:, st : st + 1],
                                    start=(st == 0),
                                    stop=(st == ST - 1),
                                )

                # contextT -> bf16 -> DRAM for AllGather
                ctxT_sb = cst.tile([P, MT, BPC], BF, tag="ctxT_sb")
                nc.vector.tensor_copy(
                    ctxT_sb[:].rearrange("p t b -> p (t b)"), ctx_ps[:]
                )
                nc.sync.dma_start(
                    ctxT_dram[:].rearrange("(t p) b -> p t b", p=P), ctxT_sb[:]
                )

            nc.gpsimd.collective_compute(
                "AllGather",
                mybir.AluOpType.bypass,
                replica_groups=[list(range(NCORES))],
                ins=[ctxT_dram.opt()],
                outs=[ctxT_all_dram.opt()],
            )

            # ---- phase 2: GRU cell, H-sharded (this core's 128-wide slice,
            #      all 64 batch rows) ----
            with (
                tc.tile_pool(name="gr_sb", bufs=2) as grs,
                tc.tile_pool(name="gr_cst", bufs=1) as grc,
                tc.tile_pool(name="gruw", bufs=6) as gruw,
                tc.tile_pool(name="g_ps", bufs=1, space="PSUM") as g_ps,
                tc.tile_pool(name="ntr_ps", bufs=1, space="PSUM") as ntr_ps,
            ):
                # xT for all 64 rows: [ctxT_all | wvT_full]
                xT_sb = grc.tile([P, KT, B], BF, tag="xT")
                for r in range(NCORES):
                    nc.sync.dma_start(
                        xT_sb[:, 0:MT, ds(r * BPC, BPC)],
                        ctxT_all_dram[ds(r * M, M), :]
                        .rearrange("(t p) b -> p t b", p=P),
                    )
                nc.sync.dma_start(
                    xT_sb[:, MT:KT, :],
                    d_wvTF.ap().rearrange("(t p) b -> p t b", p=P),
                )
                hidTF_sb = grc.tile([P, HT, B], BF, tag="hidTF")
                nc.sync.dma_start(
                    hidTF_sb[:], d_hidTF.ap().rearrange("(t p) b -> p t b", p=P)
                )
                hidfS_sb = grc.tile([B, HS], FP32, tag="hidfS")
                nc.sync.dma_start(hidfS_sb[:], d_hidfS.ap())
                gb_sb = grc.tile([B, 4 * HS], FP32, tag="gb")
                nc.sync.dma_start(gb_sb[:], d_gb.ap().broadcast_to([B, 4 * HS]))

                rps = g_ps.tile([B, HS], FP32, tag="rps")
                zps = g_ps.tile([B, HS], FP32, tag="zps")
                inps = g_ps.tile([B, HS], FP32, tag="inps")
                hnps = g_ps.tile([B, HS], FP32, tag="hnps")
                for kt in range(KT):
                    gw = gruw.tile([P, 3, HS], BF, tag="gw")
                    nc.sync.dma_start(
                        gw[:],
                        d_WihS.ap()[ts(kt, P)].rearrange("p (g c) -> p g c", g=3),
                    )
                    lhs = xT_sb[:, kt, :]
                    nc.tensor.matmul(rps[:], lhs, gw[:, 0, :],
                                     start=(kt == 0), stop=False)
                    nc.tensor.matmul(zps[:], lhs, gw[:, 1, :],
                                     start=(kt == 0), stop=False)
                    nc.tensor.matmul(inps[:], lhs, gw[:, 2, :],
                                     start=(kt == 0), stop=(kt == KT - 1))
                for kt in range(HT):
                    hw = gruw.tile([P, 3, HS], BF, tag="gw")
                    nc.sync.dma_start(
                        hw[:],
                        d_WhhS.ap()[ts(kt, P)].rearrange("p (g c) -> p g c", g=3),
                    )
                    lhs = hidTF_sb[:, kt, :]
                    nc.tensor.matmul(rps[:], lhs, hw[:, 0, :],
                                     start=False, stop=(kt == HT - 1))
                    nc.tensor.matmul(zps[:], lhs, hw[:, 1, :],
                                     start=False, stop=(kt == HT - 1))
                    nc.tensor.matmul(hnps[:], lhs, hw[:, 2, :],
                                     start=(kt == 0), stop=(kt == HT - 1))

                # r = sigm(rps + gb[0]); z = sigm(zps + gb[1]);
                # n = tanh(inps + gb[2] + r * (hnps + gb[3]))
                r_sb = grs.tile([B, HS], FP32, tag="r_sb")
                nc.vector.tensor_add(r_sb[:], rps[:], gb_sb[:, ds(0, HS)])
                nc.scalar.activation(r_sb[:], r_sb[:], SIGM)
                z_sb = grs.tile([B, HS], FP32, tag="z_sb")
                nc.vector.tensor_add(z_sb[:], zps[:], gb_sb[:, ds(HS, HS)])
                nc.scalar.activation(z_sb[:], z_sb[:], SIGM)
                hn_sb = grs.tile([B, HS], FP32, tag="hn_sb")
                nc.vector.tensor_add(hn_sb[:], hnps[:], gb_sb[:, ds(3 * HS, HS)])
                nc.vector.tensor_mul(hn_sb[:], hn_sb[:], r_sb[:])
                in_sb = grs.tile([B, HS], FP32, tag="in_sb")
                nc.vector.tensor_add(in_sb[:], inps[:], gb_sb[:, ds(2 * HS, HS)])
                nc.vector.tensor_add(in_sb[:], in_sb[:], hn_sb[:])
                n_sb = grs.tile([B, HS], FP32, tag="n_sb")
                nc.scalar.activation(n_sb[:], in_sb[:], TANH)
                # nh = n + z * (hidden_slice - n)
                dd = grs.tile([B, HS], FP32, tag="dd")
                nc.vector.tensor_sub(dd[:], hidfS_sb[:], n_sb[:])
                nc.vector.tensor_mul(dd[:], dd[:], z_sb[:])
                nh_sb = grs.tile([B, HS], FP32, tag="nh_sb")
                nc.vector.tensor_add(nh_sb[:], n_sb[:], dd[:])

                nc.sync.dma_start(d_nh.ap(), nh_sb[:])
                nhbf_sb = grs.tile([B, HS], BF, tag="nhbf")
                nc.vector.tensor_copy(nhbf_sb[:], nh_sb[:])
                ntp = ntr_ps.tile([P, B], BF, tag="ntp")
                nc.tensor.transpose(ntp[:], nhbf_sb[:], ident_bf[:B, :B])
                nhT_sb = grs.tile([P, B], BF, tag="nhT_sb")
                nc.vector.tensor_copy(nhT_sb[:], ntp[:])
                nc.sync.dma_start(nhT_dram[:], nhT_sb[:])

            nc.gpsimd.collective_compute(
                "AllGather",
                mybir.AluOpType.bypass,
                replica_groups=[list(range(NCORES))],
                ins=[nhT_dram.opt()],
                outs=[nhT_all_dram.opt()],
            )

            # ---- phase 3: vocab-sharded out projection ----
            with (
                tc.tile_pool(name="op_sb", bufs=2) as ops,
                tc.tile_pool(name="outw", bufs=3) as outw,
                tc.tile_pool(name="op_ps", bufs=1, space="PSUM") as op_ps,
            ):
                outb_sb = ops.tile([B, VS], FP32, tag="outb")
                nc.sync.dma_start(outb_sb[:], d_outb.ap().broadcast_to([B, VS]))
                nhT_all_sb = ops.tile([P, HT, B], BF, tag="nhT_all")
                nc.sync.dma_start(
                    nhT_all_sb[:],
                    nhT_all_dram[:].rearrange("(t p) b -> p t b", p=P),
                )
                NCH = 8
                CW = VS // NCH  # 500
                pps = []
                for c in range(NCH):
                    pp = op_ps.tile([B, CW], FP32, tag=f"pps{c}")
                    pps.append(pp)
                for kt in range(HT):
                    owt = outw.tile([P, VS], BF, tag="owt")
                    nc.sync.dma_start(owt[:], d_outwT.ap()[ts(kt, P)])
                    for c in range(NCH):
                        nc.tensor.matmul(
                            pps[c][:],
                            nhT_all_sb[:, kt, :],
                            owt[:, ds(c * CW, CW)],
                            start=(kt == 0),
                            stop=(kt == HT - 1),
                        )
                for c in range(NCH):
                    psb = ops.tile([B, CW], FP32, tag="psb")
                    nc.vector.tensor_add(psb[:], pps[c][:], outb_sb[:, ds(c * CW, CW)])
                    nc.sync.dma_start(d_pred.ap()[:, ds(c * CW, CW)], psb[:])

    nc.compile()
    return nc


def _bf(x):
    return np.ascontiguousarray(np.asarray(x, np.float32)).astype(BF16)


def _prep_in_maps(inputs):
    """Host-side prep: dtype casts, transposes, sharding. Returns per-core
    input dicts keyed by the kernel's DRAM tensor names."""
    f32 = lambda k: np.asarray(inputs[k], np.float32)
    word = np.asarray(inputs["word"]).astype(np.int64)
    emb = f32("emb")
    hidden = f32("hidden")
    enc = f32("encoder_outputs")

    word_vec = emb[word]                                   # [B, EMB] host gather
    enc_bf = _bf(enc)                                      # [B, S, M]
    hidT_bf = _bf(hidden.T)                                # [H, B]
    wvT_bf = _bf(word_vec.T)                               # [EMB, B]
    lin1wT = _bf(f32("lin1_w").T)
    lin2wT = _bf(f32("lin2_w").T)
    hxb = np.ascontiguousarray(
        (f32("lin1_b") + f32("lin2_b")).reshape(HT, P).T
    )                                                      # [128, HT]
    vcolT = _bf(f32("v_w")[0].reshape(HT, P).T)            # [128, HT]
    WihT = _bf(f32("W_ih").T)                              # [2560, 3072]
    WhhT = _bf(f32("W_hh").T)                              # [1024, 3072]
    b_ih, b_hh = f32("b_ih"), f32("b_hh")
    brz = b_ih[: 2 * H] + b_hh[: 2 * H]
    b_in_full = b_ih[2 * H :]
    b_hn_full = b_hh[2 * H :]
    outwT = _bf(f32("out_w").T)                            # [H, V]
    outb = f32("out_b")

    in_maps = []
    for c in range(NCORES):
        bs = slice(c * BPC, (c + 1) * BPC)
        vs = slice(c * VS, (c + 1) * VS)
        hsl = slice(c * HS, (c + 1) * HS)
        # GRU weight slice: columns [r_c | z_c | n_c] of the transposed mats
        wih_cols = np.concatenate(
            [WihT[:, hsl], WihT[:, H + c * HS : H + (c + 1) * HS],
             WihT[:, 2 * H + c * HS : 2 * H + (c + 1) * HS]], axis=1)
        whh_cols = np.concatenate(
            [WhhT[:, hsl], WhhT[:, H + c * HS : H + (c + 1) * HS],
             WhhT[:, 2 * H + c * HS : 2 * H + (c + 1) * HS]], axis=1)
        gb = np.concatenate(
            [brz[hsl], brz[H + c * HS : H + (c + 1) * HS],
             b_in_full[hsl], b_hn_full[hsl]])[None, :]
        in_maps.append(
            {
                "enc": np.ascontiguousarray(enc_bf[bs]),
                "hidT": np.ascontiguousarray(hidT_bf[:, bs]),
                "hidTF": hidT_bf,
                "hidfS": np.ascontiguousarray(hidden[:, hsl]),
                "wvTF": wvT_bf,
                "lin1wT": lin1wT,
                "lin2wT": lin2wT,
                "hxb": hxb,
                "vcolT": vcolT,
                "WihS": np.ascontiguousarray(wih_cols),
                "WhhS": np.ascontiguousarray(whh_cols),
                "gb": np.ascontiguousarray(gb),
                "outwT": np.ascontiguousarray(outwT[:, vs]),
                "outb": np.ascontiguousarray(outb[None, vs]),
            }
        )
    return in_maps


def _assemble(results):
    pred = np.concatenate([r["pred"] for r in results], axis=1)
    nh = np.concatenate([r["nh"] for r in results], axis=1)
    score = np.concatenate([r["score"] for r in results], axis=0)
    return (
        np.ascontiguousarray(pred, dtype=np.float32),
        np.ascontiguousarray(nh, dtype=np.float32),
        np.ascontiguousarray(score, dtype=np.float32),
    )


def kernel(**inputs):
    from concourse import bass_utils

    if "nc" not in _CACHE:
        _CACHE["nc"] = _build_program()
    nc = _CACHE["nc"]
    in_maps = _prep_in_maps(inputs)
    res = bass_utils.run_bass_kernel_spmd(
        nc, in_maps, core_ids=list(range(NCORES))
    )
    return _assemble(res.results)


# revision 15
# speedup vs baseline: 1.0962x; 1.0087x over previous
"""Trainium2 Bass kernel for nn_Decoder_73194832658812.

Single-step attention decoder:
  word-embedding lookup + Bahdanau additive attention over encoder memory
  + GRUCell step + vocab projection.

Sharding (8 NeuronCores):
  - data-parallel over batch (8 rows/core) for the attention phase
  - AllGather of context (bf16), then H-sharded GRU (each core computes a
    128-wide slice of new_hidden for all 64 rows)
  - AllGather of new_hidden slices, then vocab-sharded out-projection
    (each core computes 4000 of the 32000 logits for all 64 batch rows).

All matmuls run in bf16 with fp32 PSUM accumulation; softmax / GRU combine
arithmetic stays fp32. The attention batch loop is software-pipelined so the
PE stream never waits on the DVE/ACT softmax chain.
"""

import numpy as np
import ml_dtypes

BF16 = ml_dtypes.bfloat16

# problem dims (hardcoded per spec)
B, S, M, H, EMB, V = 64, 512, 2048, 1024, 512, 32000
NCORES = 8
BPC = B // NCORES          # 8 batch rows per core
VS = V // NCORES           # 4000 vocab columns per core
HS = H // NCORES           # 128 hidden slice per core (GRU shard)
XK = M + EMB               # 2560 GRU input width
P = 128
MT, HT, ET, ST = M // P, H // P, EMB // P, S // P   # 16, 8, 4, 4
KT = MT + ET               # 20 k-tiles of GRU input

_CACHE = {}


def _build_program():
    """Build + compile the per-core Bass program (SPMD: same program on all
    8 cores, per-core data differs)."""
    import concourse.bass as bass
    from concourse import bacc, mybir, tile
    from concourse.bass import ts, ds
    from concourse.masks import make_identity

    dt = mybir.dt
    FP32, BF = dt.float32, dt.bfloat16
    TANH = mybir.ActivationFunctionType.Tanh
    SIGM = mybir.ActivationFunctionType.Sigmoid
    EXP = mybir.ActivationFunctionType.Exp

    nc = bacc.Bacc(
        "TRN2",
        target_bir_lowering=False,
        debug=False,
        enable_asserts=False,
        num_devices=NCORES,
    )

    # ---- per-core DRAM I/O (p-major layouts host-prepped) ----
    d_enc = nc.dram_tensor("enc", [BPC, S, M], BF, kind="ExternalInput")
    d_hidT = nc.dram_tensor("hidT", [P, HT, BPC], BF, kind="ExternalInput")
    d_hidTF = nc.dram_tensor("hidTF", [P, HT, B], BF, kind="ExternalInput")
    d_hidfS = nc.dram_tensor("hidfS", [B, HS], FP32, kind="ExternalInput")
    d_wvTF = nc.dram_tensor("wvTF", [P, ET, B], BF, kind="ExternalInput")
    d_lin1wT = nc.dram_tensor("lin1wT", [H, H], BF, kind="ExternalInput")
    d_lin2wT = nc.dram_tensor("lin2wT", [M, H], BF, kind="ExternalInput")
    d_hxb = nc.dram_tensor("hxb", [P, HT], FP32, kind="ExternalInput")
    d_vcolT = nc.dram_tensor("vcolT", [P, HT], BF, kind="ExternalInput")
    d_WihS = nc.dram_tensor("WihS", [XK, 3 * HS], BF, kind="ExternalInput")
    d_WhhS = nc.dram_tensor("WhhS", [H, 3 * HS], BF, kind="ExternalInput")
    d_gb = nc.dram_tensor("gb", [1, 4 * HS], FP32, kind="ExternalInput")
    d_outwT = nc.dram_tensor("outwT", [H, VS], BF, kind="ExternalInput")
    d_outb = nc.dram_tensor("outb", [1, VS], FP32, kind="ExternalInput")

    d_pred = nc.dram_tensor("pred", [B, VS], FP32, kind="ExternalOutput")
    d_nh = nc.dram_tensor("nh", [B, HS], FP32, kind="ExternalOutput")
    d_score = nc.dram_tensor("score", [BPC, S], FP32, kind="ExternalOutput")

    with tile.TileContext(nc) as tc:
        with (
            tc.tile_pool(name="const", bufs=1) as cst,
            tc.tile_pool(name="wres", bufs=1) as wres,
            tc.tile_pool(name="outw", bufs=4) as outw,
            tc.tile_pool(name="gruw", bufs=1) as gruw,
            tc.tile_pool(name="dram", bufs=1, space="DRAM") as drp,
        ):
            ident_bf = cst.tile([P, P], BF, tag="ident_bf")
            make_identity(nc, ident_bf[:])
            ident_f = cst.tile([P, P], FP32, tag="ident_f")
            make_identity(nc, ident_f[:])

            lin2w_sb = wres.tile([P, MT, H], BF, tag="lin2w")
            nc.sync.dma_start(
                lin2w_sb[:], d_lin2wT.ap().rearrange("(mt p) h -> p mt h", p=P)
            )
            hidT_sb = cst.tile([P, HT, BPC], BF, tag="hidT")
            nc.sync.dma_start(hidT_sb[:], d_hidT.ap())
            hxb_sb = cst.tile([P, HT], FP32, tag="hxb")
            nc.sync.dma_start(hxb_sb[:], d_hxb.ap())
            vcolT_sb = cst.tile([P, HT], BF, tag="vcolT")
            nc.sync.dma_start(vcolT_sb[:], d_vcolT.ap())
            hxbias_sb = cst.tile([P, HT * BPC], FP32, tag="hxbias")

            # DRAM staging for the two AllGathers
            ctxN_dram = drp.tile([BPC, M], BF, tag="ctxN_in")
            ctxN_all_dram = drp.tile([B, M], BF, tag="ctxN_all",
                                     addr_space="Shared")
            nhT_dram = drp.tile([P, B], BF, tag="nhT_in")
            nhT_all_dram = drp.tile([NCORES * P, B], BF, tag="nhT_all",
                                    addr_space="Shared")

            # GRU weight tiles (all prefetched; slices of W_ih/W_hh for this
            # core's 128-wide gate slice)
            gw_tiles = []
            for kt in range(KT):
                gwi = gruw.tile([P, 3, HS], BF, tag=f"gw{kt}")
                nc.sync.dma_start(
                    gwi[:],
                    d_WihS.ap()[ts(kt, P)].rearrange("p (g c) -> p g c", g=3),
                )
                gw_tiles.append(gwi)
            hw_tiles = []
            for kt in range(HT):
                hwi = gruw.tile([P, 3, HS], BF, tag=f"hw{kt}")
                nc.sync.dma_start(
                    hwi[:],
                    d_WhhS.ap()[ts(kt, P)].rearrange("p (g c) -> p g c", g=3),
                )
                hw_tiles.append(hwi)

            # ---- prologue: h_xT = (hidden @ lin1_w.T).T + (lin1_b+lin2_b) ----
            with (
                tc.tile_pool(name="pro", bufs=1) as pro,
                tc.tile_pool(name="pro_ps", bufs=2, space="PSUM") as pro_ps,
            ):
                lin1w_sb = pro.tile([P, HT, H], BF, tag="lin1w")
                nc.sync.dma_start(
                    lin1w_sb[:], d_lin1wT.ap().rearrange("(t p) h -> p t h", p=P)
                )
                hx_sb = pro.tile([BPC, H], FP32, tag="hx")
                for ch in range(2):
                    hx_ps = pro_ps.tile([BPC, 512], FP32, tag="hx_ps")
                    for kt in range(HT):
                        nc.tensor.matmul(
                            hx_ps[:],
                            hidT_sb[:, kt, :],
                            lin1w_sb[:, kt, ds(ch * 512, 512)],
                            start=(kt == 0),
                            stop=(kt == HT - 1),
                        )
                    nc.vector.tensor_copy(hx_sb[:, ds(ch * 512, 512)], hx_ps[:])
                for t in range(HT):
                    tp_ps = pro_ps.tile([P, BPC], FP32, tag="tp_ps")
                    nc.tensor.transpose(
                        tp_ps[:], hx_sb[:, ds(t * P, P)], ident_f[:BPC, :BPC]
                    )
                    nc.vector.tensor_add(
                        hxbias_sb[:, ds(t * BPC, BPC)],
                        tp_ps[:],
                        hxb_sb[:, t : t + 1].broadcast_to([P, BPC]),
                    )

            # ---- phase 1: attention, software-pipelined over batch rows ----
            # stage A(b):   DMA encT/encN, h_m matmuls, tanh
            # stage B(b-1): e matvec, softmax chain
            # stage C(b-2): score transposes, context matmuls
            with (
                tc.tile_pool(name="encT", bufs=2) as encTp,
                tc.tile_pool(name="encN", bufs=3) as encNp,
                tc.tile_pool(name="tanh", bufs=2) as tanhp,
                tc.tile_pool(name="attn_smA", bufs=1) as smA,
                tc.tile_pool(name="attn_smB", bufs=2) as smB,
                tc.tile_pool(name="hm_ps", bufs=3, space="PSUM") as hm_ps,
                tc.tile_pool(name="e_ps", bufs=2, space="PSUM") as e_ps,
                tc.tile_pool(name="ctx_ps", bufs=2, space="PSUM") as ctx_psp,
                tc.tile_pool(name="tr_ps", bufs=1, space="PSUM") as tr_ps,
            ):
                stA, stB = {}, {}
                for step in range(BPC + 2):
                    if step < BPC:
                        b = step
                        encT = encTp.tile([P, MT, S], BF, tag="encT")
                        for mt in range(MT):
                            nc.sync.dma_start_transpose(
                                encT[:, mt, :], d_enc.ap()[b, :, ds(mt * P, P)]
                            )
                        encN = encNp.tile([P, ST, M], BF, tag="encN")
                        nc.sync.dma_start(
                            encN[:],
                            d_enc.ap()[b].rearrange("(st p) m -> p st m", p=P),
                        )
                        t_sb = tanhp.tile([P, HT, S], BF, tag="t_sb")
                        for ht in range(HT):
                            hm = hm_ps.tile([P, S], FP32, tag="hm")
                            for mt in range(MT):
                                nc.tensor.matmul(
                                    hm[:],
                                    lin2w_sb[:, mt, ds(ht * P, P)],
                                    encT[:, mt, :],
                                    start=(mt == 0),
                                    stop=(mt == MT - 1),
                                )
                            nc.scalar.activation(
                                t_sb[:, ht, :],
                                hm[:],
                                TANH,
                                bias=hxbias_sb[:, ht * BPC + b : ht * BPC + b + 1],
                            )
                        stA[b] = (encN, t_sb)

                    if 1 <= step <= BPC:
                        p_ = step - 1
                        encN, t_sb = stA.pop(p_)
                        e_psum = e_ps.tile([1, S], FP32, tag="e")
                        for ht in range(HT):
                            nc.tensor.matmul(
                                e_psum[:],
                                vcolT_sb[:, ht : ht + 1],
                                t_sb[:, ht, :],
                                start=(ht == 0),
                                stop=(ht == HT - 1),
                            )
                        negmax = smA.tile([1, 1], FP32, tag="negmax")
                        nc.vector.tensor_reduce(
                            negmax[:], e_psum[:], axis=mybir.AxisListType.X,
                            op=mybir.AluOpType.max, negate=True,
                        )
                        prob = smA.tile([1, S], FP32, tag="prob")
                        ssum = smA.tile([1, 1], FP32, tag="ssum")
                        nc.scalar.activation(
                            prob[:], e_psum[:], EXP, bias=negmax[:], accum_out=ssum[:]
                        )
                        rinv = smA.tile([1, 1], FP32, tag="rinv")
                        nc.vector.reciprocal(rinv[:], ssum[:])
                        score_sb = smA.tile([1, S], FP32, tag="score")
                        nc.vector.tensor_scalar_mul(score_sb[:], prob[:], rinv[:])
                        nc.sync.dma_start(d_score.ap()[p_ : p_ + 1, :], score_sb[:])
                        score_bf = smB.tile([1, S], BF, tag="score_bf")
                        nc.vector.tensor_copy(score_bf[:], score_sb[:])
                        stB[p_] = (encN, score_bf)

                    if step >= 2:
                        q = step - 2
                        encN, score_bf = stB.pop(q)
                        score_colT = smB.tile([P, ST], BF, tag="score_colT")
                        for st in range(ST):
                            trp = tr_ps.tile([P, 1], BF, tag="trp")
                            nc.tensor.transpose(
                                trp[:], score_bf[:, ds(st * P, P)], ident_bf[:1, :1]
                            )
                            nc.vector.tensor_copy(score_colT[:, st : st + 1], trp[:])
                        # context row (natural layout): ctx[q, m] += score . enc
                        ctxrow = smB.tile([1, M], BF, tag="ctxrow")
                        for mc in range(4):
                            ctxr = ctx_psp.tile([1, 512], FP32, tag="ctxr")
                            for st in range(ST):
                                nc.tensor.matmul(
                                    ctxr[:],
                                    score_colT[:, st : st + 1],
                                    encN[:, st, ds(mc * 512, 512)],
                                    start=(st == 0),
                                    stop=(st == ST - 1),
                                )
                            nc.vector.tensor_copy(
                                ctxrow[:, ds(mc * 512, 512)], ctxr[:]
                            )
                        nc.sync.dma_start(ctxN_dram[q : q + 1, :], ctxrow[:])

            nc.gpsimd.collective_compute(
                "AllGather",
                mybir.AluOpType.bypass,
                replica_groups=[list(range(NCORES))],
                ins=[ctxN_dram.opt()],
                outs=[ctxN_all_dram.opt()],
            )

            # ---- phase 2: GRU cell, H-sharded (this core's 128-wide slice,
            #      all 64 batch rows) ----
            with (
                tc.tile_pool(name="gr_sb", bufs=2) as grs,
                tc.tile_pool(name="gr_cst", bufs=1) as grc,
                tc.tile_pool(name="g_ps", bufs=1, space="PSUM") as g_ps,
                tc.tile_pool(name="ntr_ps", bufs=1, space="PSUM") as ntr_ps,
            ):
                # xT k-tiles for the GRU input: one XBAR transpose of the
                # gathered natural-layout context + word-vec (host p-major)
                xT_sb = grc.tile([P, MT, B], BF, tag="xT")
                nc.sync.dma_start_transpose(xT_sb[:], ctxN_all_dram[:])
                wv_sb = grc.tile([P, ET, B], BF, tag="wv")
                nc.sync.dma_start(wv_sb[:], d_wvTF.ap())
                hidTF_sb = grc.tile([P, HT, B], BF, tag="hidTF")
                nc.sync.dma_start(hidTF_sb[:], d_hidTF.ap())
                hidfS_sb = grc.tile([B, HS], FP32, tag="hidfS")
                nc.sync.dma_start(hidfS_sb[:], d_hidfS.ap())
                gb_sb = grc.tile([B, 4 * HS], FP32, tag="gb")
                nc.sync.dma_start(gb_sb[:], d_gb.ap().broadcast_to([B, 4 * HS]))

                rps = g_ps.tile([B, HS], FP32, tag="rps")
                zps = g_ps.tile([B, HS], FP32, tag="zps")
                inps = g_ps.tile([B, HS], FP32, tag="inps")
                hnps = g_ps.tile([B, HS], FP32, tag="hnps")
                # W_hh part first: depends only on inputs, overlaps AllGather
                for kt in range(HT):
                    lhs = hidTF_sb[:, kt, :]
                    hwi = hw_tiles[kt]
                    nc.tensor.matmul(rps[:], lhs, hwi[:, 0, :],
                                     start=(kt == 0), stop=False)
                    nc.tensor.matmul(zps[:], lhs, hwi[:, 1, :],
                                     start=(kt == 0), stop=False)
                    nc.tensor.matmul(hnps[:], lhs, hwi[:, 2, :],
                                     start=(kt == 0), stop=(kt == HT - 1))
                for kt in range(KT):
                    lhs = xT_sb[:, kt, :] if kt < MT else wv_sb[:, kt - MT, :]
                    gwi = gw_tiles[kt]
                    nc.tensor.matmul(rps[:], lhs, gwi[:, 0, :],
                                     start=False, stop=(kt == KT - 1))
                    nc.tensor.matmul(zps[:], lhs, gwi[:, 1, :],
                                     start=False, stop=(kt == KT - 1))
                    nc.tensor.matmul(inps[:], lhs, gwi[:, 2, :],
                                     start=(kt == 0), stop=(kt == KT - 1))

                # r = sigm(rps + gb[0]); z = sigm(zps + gb[1]);
                # n = tanh(inps + gb[2] + r * (hnps + gb[3]))
                r_sb = grs.tile([B, HS], FP32, tag="r_sb")
                nc.vector.tensor_add(r_sb[:], rps[:], gb_sb[:, ds(0, HS)])
                nc.scalar.activation(r_sb[:], r_sb[:], SIGM)
                z_sb = grs.tile([B, HS], FP32, tag="z_sb")
                nc.vector.tensor_add(z_sb[:], zps[:], gb_sb[:, ds(HS, HS)])
                nc.scalar.activation(z_sb[:], z_sb[:], SIGM)
                hn_sb = grs.tile([B, HS], FP32, tag="hn_sb")
                nc.vector.tensor_add(hn_sb[:], hnps[:], gb_sb[:, ds(3 * HS, HS)])
                nc.vector.tensor_mul(hn_sb[:], hn_sb[:], r_sb[:])
                in_sb = grs.tile([B, HS], FP32, tag="in_sb")
                nc.vector.tensor_add(in_sb[:], inps[:], gb_sb[:, ds(2 * HS, HS)])
                nc.vector.tensor_add(in_sb[:], in_sb[:], hn_sb[:])
                n_sb = grs.tile([B, HS], FP32, tag="n_sb")
                nc.scalar.activation(n_sb[:], in_sb[:], TANH)
                # nh = n + z * (hidden_slice - n)
                dd = grs.tile([B, HS], FP32, tag="dd")
                nc.vector.tensor_sub(dd[:], hidfS_sb[:], n_sb[:])
                nc.vector.tensor_mul(dd[:], dd[:], z_sb[:])
                nh_sb = grs.tile([B, HS], FP32, tag="nh_sb")
                nc.vector.tensor_add(nh_sb[:], n_sb[:], dd[:])

                nc.sync.dma_start(d_nh.ap(), nh_sb[:])
                nhbf_sb = grs.tile([B, HS], BF, tag="nhbf")
                nc.vector.tensor_copy(nhbf_sb[:], nh_sb[:])
                ntp = ntr_ps.tile([P, B], BF, tag="ntp")
                nc.tensor.transpose(ntp[:], nhbf_sb[:], ident_bf[:B, :B])
                nhT_sb = grs.tile([P, B], BF, tag="nhT_sb")
                nc.vector.tensor_copy(nhT_sb[:], ntp[:])
                nc.sync.dma_start(nhT_dram[:], nhT_sb[:])

            nc.gpsimd.collective_compute(
                "AllGather",
                mybir.AluOpType.bypass,
                replica_groups=[list(range(NCORES))],
                ins=[nhT_dram.opt()],
                outs=[nhT_all_dram.opt()],
            )

            # ---- phase 3: vocab-sharded out projection ----
            with (
                tc.tile_pool(name="op_sb", bufs=2) as ops,
                tc.tile_pool(name="op_ps", bufs=1, space="PSUM") as op_ps,
            ):
                outb_sb = ops.tile([B, VS], FP32, tag="outb")
                nc.sync.dma_start(outb_sb[:], d_outb.ap().broadcast_to([B, VS]))
                nhT_all_sb = ops.tile([P, NCORES, B], BF, tag="nhT_all")
                nc.sync.dma_start(
                    nhT_all_sb[:],
                    nhT_all_dram[:].rearrange("(r p) b -> p r b", p=P),
                )
                NCH = 8
                CW = VS // NCH  # 500
                pps = []
                for c in range(NCH):
                    pp = op_ps.tile([B, CW], FP32, tag=f"pps{c}")
                    pps.append(pp)
                for kt in range(HT):
                    owt = outw.tile([P, VS], BF, tag="owt")
                    nc.sync.dma_start(owt[:], d_outwT.ap()[ts(kt, P)])
                    for c in range(NCH):
                        nc.tensor.matmul(
                            pps[c][:],
                            nhT_all_sb[:, kt, :],
                            owt[:, ds(c * CW, CW)],
                            start=(kt == 0),
                            stop=(kt == HT - 1),
                        )
                for c in range(NCH):
                    psb = ops.tile([B, CW], FP32, tag="psb")
                    nc.vector.tensor_add(psb[:], pps[c][:], outb_sb[:, ds(c * CW, CW)])
                    nc.sync.dma_start(d_pred.ap()[:, ds(c * CW, CW)], psb[:])

    nc.compile()
    return nc


def _bf(x):
    return np.ascontiguousarray(np.asarray(x, np.float32)).astype(BF16)


def _pmaj(arr2d, nt):
    """[nt*128, C] -> p-major [128, nt, C] (contiguous)."""
    return np.ascontiguousarray(
        arr2d.reshape(nt, P, arr2d.shape[1]).transpose(1, 0, 2)
    )


def _prep_in_maps(inputs):
    """Host-side prep: dtype casts, transposes, sharding. Returns per-core
    input dicts keyed by the kernel's DRAM tensor names."""
    f32 = lambda k: np.asarray(inputs[k], np.float32)
    word = np.asarray(inputs["word"]).astype(np.int64)
    emb = f32("emb")
    hidden = f32("hidden")
    enc = f32("encoder_outputs")

    word_vec = emb[word]                                   # [B, EMB] host gather
    enc_bf = _bf(enc)                                      # [B, S, M]
    hidT_bf = _bf(hidden.T)                                # [H, B]
    hidTF_pm = _pmaj(hidT_bf, HT)                          # [128, 8, 64]
    wvTF_pm = _pmaj(_bf(word_vec.T), ET)                   # [128, 4, 64]
    lin1wT = _bf(f32("lin1_w").T)
    lin2wT = _bf(f32("lin2_w").T)
    hxb = np.ascontiguousarray(
        (f32("lin1_b") + f32("lin2_b")).reshape(HT, P).T
    )                                                      # [128, HT]
    vcolT = _bf(f32("v_w")[0].reshape(HT, P).T)            # [128, HT]
    WihT = _bf(f32("W_ih").T)                              # [2560, 3072]
    WhhT = _bf(f32("W_hh").T)                              # [1024, 3072]
    b_ih, b_hh = f32("b_ih"), f32("b_hh")
    brz = b_ih[: 2 * H] + b_hh[: 2 * H]
    b_in_full = b_ih[2 * H :]
    b_hn_full = b_hh[2 * H :]
    outwT = _bf(f32("out_w").T)                            # [H, V]
    outb = f32("out_b")

    in_maps = []
    for c in range(NCORES):
        bs = slice(c * BPC, (c + 1) * BPC)
        vs = slice(c * VS, (c + 1) * VS)
        hsl = slice(c * HS, (c + 1) * HS)
        wih_cols = np.concatenate(
            [WihT[:, hsl], WihT[:, H + c * HS : H + (c + 1) * HS],
             WihT[:, 2 * H + c * HS : 2 * H + (c + 1) * HS]], axis=1)
        whh_cols = np.concatenate(
            [WhhT[:, hsl], WhhT[:, H + c * HS : H + (c + 1) * HS],
             WhhT[:, 2 * H + c * HS : 2 * H + (c + 1) * HS]], axis=1)
        gb = np.concatenate(
            [brz[hsl], brz[H + c * HS : H + (c + 1) * HS],
             b_in_full[hsl], b_hn_full[hsl]])[None, :]
        in_maps.append(
            {
                "enc": np.ascontiguousarray(enc_bf[bs]),
                "hidT": _pmaj(np.ascontiguousarray(hidT_bf[:, bs]), HT),
                "hidTF": hidTF_pm,
                "hidfS": np.ascontiguousarray(hidden[:, hsl]),
                "wvTF": wvTF_pm,
                "lin1wT": lin1wT,
                "lin2wT": lin2wT,
                "hxb": hxb,
                "vcolT": vcolT,
                "WihS": np.ascontiguousarray(wih_cols),
                "WhhS": np.ascontiguousarray(whh_cols),
                "gb": np.ascontiguousarray(gb),
                "outwT": np.ascontiguousarray(outwT[:, vs]),
                "outb": np.ascontiguousarray(outb[None, vs]),
            }
        )
    return in_maps


def _assemble(results):
    pred = np.concatenate([r["pred"] for r in results], axis=1)
    nh = np.concatenate([r["nh"] for r in results], axis=1)
    score = np.concatenate([r["score"] for r in results], axis=0)
    return (
        np.ascontiguousarray(pred, dtype=np.float32),
        np.ascontiguousarray(nh, dtype=np.float32),
        np.ascontiguousarray(score, dtype=np.float32),
    )


def kernel(**inputs):
    from concourse import bass_utils

    if "nc" not in _CACHE:
        _CACHE["nc"] = _build_program()
    nc = _CACHE["nc"]
    in_maps = _prep_in_maps(inputs)
    res = bass_utils.run_bass_kernel_spmd(
        nc, in_maps, core_ids=list(range(NCORES))
    )
    return _assemble(res.results)
